# revision 6
# baseline (speedup 1.0000x reference)
"""Balanced EMD loss kernel for Trainium2 (8 NeuronCores, data parallel).

Math (per sample, classes w = 1..10):
    score = sum(pt * w);  var = sum(pt * (w - score)^2) = Z2 - Z1^2  (S0 ~= 1)
    cdf_diff = cumsum(pe) - cumsum(pt) = cumsum(pe - pt)
    emd = sqrt(mean(cdf_diff^2));  loss = sum(emd / var) / B

Layout: samples are distributed over 128 partitions; each partition holds a
contiguous run of samples, 10 classes contiguous in the free dim.  Per-sample
reductions over the 10 classes run as masked scans (tensor_tensor_scan with a
periodic multiplicative pattern) and pool_avg; the weighted moments Z1, Z2
fall out of scans with patterns r_j = (j-1)/j and r_j^2, whose state at the
last class equals Z1/10 and Z2/100.
"""

import numpy as np

P = 128          # SBUF partitions
C = 10           # classes
K = 392          # samples per partition per tile
NT = 10          # tiles
KT = K * NT      # samples per partition per core
SHARD = P * KT   # 501760 padded rows per core
NCORES = 8
PAD_VAL = 0.1    # pt == pe == 0.1 -> emd == 0 -> zero loss contribution

_CACHE = {}


def _build_nc(k=K, nt=NT):
    import concourse.bass as bass
    import concourse.tile as tile
    from concourse import bacc, mybir

    dt = mybir.dt.float32
    Alu = mybir.AluOpType
    F = k * C

    nc = bacc.Bacc("TRN2")
    pt_d = nc.dram_tensor("pt", [P, k * nt, C], dt, kind="ExternalInput").ap()
    pe_d = nc.dram_tensor("pe", [P, k * nt, C], dt, kind="ExternalInput").ap()
    msk_d = nc.dram_tensor("mask01", [P, F], dt, kind="ExternalInput").ap()
    rp1_d = nc.dram_tensor("rp1", [P, F], dt, kind="ExternalInput").ap()
    rp2_d = nc.dram_tensor("rp2", [P, F], dt, kind="ExternalInput").ap()
    out_d = nc.dram_tensor("out", [P, nt], dt, kind="ExternalOutput").ap()

    with tile.TileContext(nc) as tc:
        with (
            tc.tile_pool(name="consts", bufs=1) as cpool,
            tc.tile_pool(name="ins", bufs=2) as ipool,
            tc.tile_pool(name="work", bufs=2) as wpool,
            tc.tile_pool(name="small", bufs=2) as spool,
            tc.tile_pool(name="outp", bufs=1) as opool,
        ):
            cmask = cpool.tile([P, F], dt, tag="cmask")
            nc.sync.dma_start(cmask[:], msk_d[:])
            crp1 = cpool.tile([P, F], dt, tag="crp1")
            nc.sync.dma_start(crp1[:], rp1_d[:])
            crp2 = cpool.tile([P, F], dt, tag="crp2")
            nc.sync.dma_start(crp2[:], rp2_d[:])

            acc = opool.tile([P, nt], dt, tag="acc")

            for i in range(nt):
                ptt = ipool.tile([P, F], dt, tag="ptt")
                nc.sync.dma_start(
                    ptt[:].rearrange("p (k c) -> p k c", c=C),
                    pt_d[:, bass.ts(i, k), :],
                )
                pet = ipool.tile([P, F], dt, tag="pet")
                nc.sync.dma_start(
                    pet[:].rearrange("p (k c) -> p k c", c=C),
                    pe_d[:, bass.ts(i, k), :],
                )

                # q = pe - pt  (in place on the pe tile)
                nc.vector.tensor_sub(pet[:], pet[:], ptt[:])

                # cdf_diff: state = mask*state + q, resets at each sample start
                cdf = wpool.tile([P, F], dt, tag="cdf")
                nc.vector.tensor_tensor_scan(
                    cdf[:], cmask[:], pet[:], 0.0, op0=Alu.mult, op1=Alu.add
                )

                # square on the scalar engine, in place
                nc.scalar.square(cdf[:], cdf[:])

                # ssqm = sum over classes of cdf_diff^2
                ssqm = spool.tile([P, k], dt, tag="ssqm")
                nc.vector.tensor_reduce(
                    ssqm[:],
                    cdf[:].rearrange("p (k c) -> p k c", c=C),
                    axis=mybir.AxisListType.X,
                    op=Alu.add,
                )

                # weighted moment scans over pt; outputs reuse the spent
                # q tile (pet) and then pt itself (in place, last use)
                nc.vector.tensor_tensor_scan(
                    pet[:], crp2[:], ptt[:], 0.0, op0=Alu.mult, op1=Alu.add
                )
                nc.vector.tensor_tensor_scan(
                    ptt[:], crp1[:], ptt[:], 0.0, op0=Alu.mult, op1=Alu.add
                )
                z1 = ptt[:].rearrange("p (k c) -> p k c", c=C)[:, :, C - 1]
                z2 = pet[:].rearrange("p (k c) -> p k c", c=C)[:, :, C - 1]

                # var = 100*z2 - 100*z1^2   (z1 = Z1/10, z2 = Z2/100)
                tv = spool.tile([P, k], dt, tag="tv")
                nc.vector.scalar_tensor_tensor(
                    tv[:], z1, -100.0, z1, op0=Alu.mult, op1=Alu.mult
                )
                # var then weight, in place on the tv tile
                nc.vector.scalar_tensor_tensor(
                    tv[:], z2, 100.0, tv[:], op0=Alu.mult, op1=Alu.add
                )
                nc.vector.reciprocal_approx_fast(tv[:], tv[:])

                # emd = sqrt(ssq/10) via the activation's free input scale,
                # in place on ssqm
                nc.scalar.activation(
                    ssqm[:], ssqm[:], mybir.ActivationFunctionType.Sqrt, scale=0.1
                )

                # acc[:, i] = sum_k emd * wgt
                # (tensor_tensor_reduce crashes this runtime; use mul+reduce)
                nc.vector.tensor_mul(tv[:], ssqm[:], tv[:])
                nc.vector.tensor_reduce(
                    acc[:, i : i + 1], tv[:], axis=mybir.AxisListType.X, op=Alu.add
                )

            nc.sync.dma_start(out_d[:], acc[:])

    nc.compile()
    return nc


def _consts(k=K):
    F = k * C
    m01 = np.tile(np.array([0.0] + [1.0] * (C - 1), np.float32), k)
    r1 = np.zeros(C, np.float64)
    r1[1:] = np.arange(1, C) / np.arange(2, C + 1)
    r1 = r1.astype(np.float32)
    r2 = (r1 * r1).astype(np.float32)
    rp1 = np.tile(r1, k)
    rp2 = np.tile(r2, k)
    return (
        np.ascontiguousarray(np.broadcast_to(m01, (P, F))),
        np.ascontiguousarray(np.broadcast_to(rp1, (P, F))),
        np.ascontiguousarray(np.broadcast_to(rp2, (P, F))),
    )


def _shards(x, per, shard_rows):
    out = []
    for i in range(NCORES):
        s = x[i * per : (i + 1) * per]
        pad = shard_rows - s.shape[0]
        if pad:
            s = np.concatenate([s, np.full((pad, C), PAD_VAL, x.dtype)], axis=0)
        out.append(np.ascontiguousarray(s.reshape(P, shard_rows // P, C)))
    return out


def kernel(p_target: np.ndarray, p_estimate: np.ndarray) -> np.ndarray:
    from concourse.bass_utils import run_bass_kernel_spmd

    if "nc" not in _CACHE:
        _CACHE["nc"] = _build_nc()
    nc = _CACHE["nc"]

    B = p_target.shape[0]
    per = B // NCORES
    mask_full, rp1_full, rp2_full = _consts()
    pt_sh = _shards(np.asarray(p_target, np.float32), per, SHARD)
    pe_sh = _shards(np.asarray(p_estimate, np.float32), per, SHARD)

    in_maps = [
        {
            "pt": pt_sh[i],
            "pe": pe_sh[i],
            "mask01": mask_full,
            "rp1": rp1_full,
            "rp2": rp2_full,
        }
        for i in range(NCORES)
    ]
    res = run_bass_kernel_spmd(nc, in_maps, core_ids=list(range(NCORES)))
    total = sum(
        res.results[i]["out"].astype(np.float64).sum() for i in range(NCORES)
    )
    return np.float32(total / B)


# revision 13
# speedup vs baseline: 1.7350x; 1.7350x over previous
"""Balanced EMD loss kernel for Trainium2 (8 NeuronCores, data parallel).

Math (per sample, classes w = 1..10):
    score = sum(pt * w);  var = sum(pt * (w - score)^2) = Z2 - Z1^2  (S0 ~= 1)
    cdf_diff = cumsum(pe) - cumsum(pt) = cumsum(pe - pt)
    emd = sqrt(mean(cdf_diff^2));  loss = sum(emd / var) / B

Layout: samples distributed over 128 partitions; each partition holds a
contiguous run of samples, 10 classes contiguous in the free dim.

Engine split per tile:
  VectorE: q = pe - pt; masked scan (per-sample cumsum via a periodic 0/1
           multiplicative reset pattern); per-sample reduce of cdf^2;
           small finishing ops (var, 1/var, loss accumulate).
  ScalarE: square of the cdf (in place) and PSUM->SBUF moves for the
           TensorE moment pipeline; final sqrt.
  TensorE: weighted moments Z1 = sum(pt*w), Z2 = sum(pt*w^2): transpose
           [128,120] chunks to class-on-partition, block-diagonal [120,24]
           matmul, transpose the [24,128] results back to a dense
           [128, samples*2] layout.
"""

import numpy as np

P = 128          # SBUF partitions
C = 10           # classes
K = 396          # samples per partition per tile (multiple of 12)
NT = 10          # tiles
KT = K * NT      # samples per partition per core
SHARD = P * KT   # padded rows per core
NCORES = 8
PAD_VAL = 0.1    # pt == pe == 0.1 -> emd == 0 -> zero loss contribution

SLOT = 12        # samples per transpose chunk (120 = SLOT*C free positions)
GCH = 3          # chunks per matmul group (PSUM bank holds 3*128 = 384 cols)

_CACHE = {}


def _build_nc(k=K, nt=NT):
    import concourse.bass as bass
    import concourse.tile as tile
    from concourse import bacc, mybir

    dt = mybir.dt.float32
    dth = mybir.dt.float16
    Alu = mybir.AluOpType
    F = k * C
    n_chunk = k // SLOT              # transpose chunks per tile
    n_group = n_chunk // GCH         # matmul groups per tile
    CW = SLOT * C                    # 120 free positions per chunk
    GW = GCH * P                     # matmul group column count (384)
    MW = GCH * 2 * SLOT              # momd free elems per group (72)

    nc = bacc.Bacc("TRN2")
    pt_d = nc.dram_tensor("pt", [P, k * nt, C], dth, kind="ExternalInput").ap()
    pe_d = nc.dram_tensor("pe", [P, k * nt, C], dth, kind="ExternalInput").ap()
    msk_d = nc.dram_tensor("mask01", [P, F], dth, kind="ExternalInput").ap()
    w_d = nc.dram_tensor("wst", [CW, 2 * SLOT], dth, kind="ExternalInput").ap()
    id_d = nc.dram_tensor("ident", [P, P], dth, kind="ExternalInput").ap()
    out_d = nc.dram_tensor("out", [P, nt], dt, kind="ExternalOutput").ap()

    with tile.TileContext(nc) as tc:
        with (
            tc.tile_pool(name="consts", bufs=1) as cpool,
            tc.tile_pool(name="ins", bufs=3) as ipool,
            tc.tile_pool(name="mm", bufs=3) as mpool,
            tc.tile_pool(name="small", bufs=2) as spool,
            tc.tile_pool(name="ps1", bufs=2, space="PSUM") as ppool1,
            tc.tile_pool(name="ps2", bufs=2, space="PSUM") as ppool2,
            tc.tile_pool(name="ps3", bufs=2, space="PSUM") as ppool3,
            tc.tile_pool(name="outp", bufs=1) as opool,
        ):
            cmask = cpool.tile([P, F], dth, tag="cmask")
            nc.sync.dma_start(cmask[:], msk_d[:])
            wst = cpool.tile([CW, 2 * SLOT], dth, tag="wst")
            nc.sync.dma_start(wst[:], w_d[:])
            ident = cpool.tile([P, P], dth, tag="ident")
            nc.sync.dma_start(ident[:], id_d[:])

            acc = opool.tile([P, nt], dt, tag="acc")

            for i in range(nt):
                ptt = ipool.tile([P, F], dth, tag="ptt")
                nc.sync.dma_start(
                    ptt[:].rearrange("p (k c) -> p k c", c=C),
                    pt_d[:, bass.ts(i, k), :],
                )
                pet = ipool.tile([P, F], dth, tag="pet")
                nc.sync.dma_start(
                    pet[:].rearrange("p (k c) -> p k c", c=C),
                    pe_d[:, bass.ts(i, k), :],
                )

                # ---- VectorE cdf pipeline ----
                # q = pe - pt  (in place on the pe tile)
                nc.vector.tensor_sub(pet[:], pet[:], ptt[:])
                # per-sample cumsum: state = mask*state + q, in place
                nc.vector.tensor_tensor_scan(
                    pet[:], cmask[:], pet[:], 0.0, op0=Alu.mult, op1=Alu.add
                )
                # square on the scalar engine, in place
                nc.scalar.square(pet[:], pet[:])
                # ssq = sum over classes of cdf_diff^2
                ssqm = spool.tile([P, k], dt, tag="ssqm")
                nc.vector.tensor_reduce(
                    ssqm[:],
                    pet[:].rearrange("p (k c) -> p k c", c=C),
                    axis=mybir.AxisListType.X,
                    op=Alu.add,
                )

                # ---- TensorE moment pipeline over pt ----
                # transpose [128,120] chunks to class-on-partition, then
                # matmul with the chunk as STATIONARY and the block-diag
                # weight matrix as moving: out = sb_chunk^T @ wst =
                # [128 samples, 24] -- moments, already dense.
                momd = mpool.tile([P, 2 * k], dt, tag="momd")
                n_half = (n_chunk + 1) // 2  # chunks in first PSUM bank
                mdp_a = ppool2.tile([P, n_half * 2 * SLOT], dt, tag="mdp_a")
                mdp_b = ppool3.tile(
                    [P, (n_chunk - n_half) * 2 * SLOT], dt, tag="mdp_b"
                )
                for g in range(n_group):
                    pst = ppool1.tile([CW, GW], dth, tag="pst")
                    for j in range(GCH):
                        ch = g * GCH + j
                        nc.tensor.transpose(
                            pst[:, bass.ts(j, P)],
                            ptt[:, bass.ts(ch, CW)],
                            ident[:],
                        )
                    sb = mpool.tile([CW, GW], dth, tag="sb")
                    nc.scalar.copy(sb[:], pst[:])
                    for j in range(GCH):
                        ch = g * GCH + j
                        dst = (
                            mdp_a[:, bass.ts(ch, 2 * SLOT)]
                            if ch < n_half
                            else mdp_b[:, bass.ts(ch - n_half, 2 * SLOT)]
                        )
                        nc.tensor.matmul(
                            dst, sb[:, bass.ts(j, P)], wst[:],
                            start=True, stop=True,
                        )
                nc.scalar.copy(momd[:, : n_half * 2 * SLOT], mdp_a[:])
                nc.scalar.copy(momd[:, n_half * 2 * SLOT :], mdp_b[:])

                # ---- finishing ----
                # momd free layout: (group, chunk, slot, mtype) -> sample
                # k = 12*(3g + j) + slot, mtype 0 -> Z1/10, 1 -> Z2/100
                z1 = momd[:].rearrange("p (k m) -> p k m", m=2)[:, :, 0]
                z2 = momd[:].rearrange("p (k m) -> p k m", m=2)[:, :, 1]
                tv = spool.tile([P, k], dt, tag="tv")
                # var = 256*z2 - 256*z1^2   (z1 = Z1/16, z2 = Z2/256)
                nc.vector.scalar_tensor_tensor(
                    tv[:], z1, -256.0, z1, op0=Alu.mult, op1=Alu.mult
                )
                nc.vector.scalar_tensor_tensor(
                    tv[:], z2, 256.0, tv[:], op0=Alu.mult, op1=Alu.add
                )
                nc.vector.reciprocal_approx_fast(tv[:], tv[:])
                # emd = sqrt(ssq/10), in place on ssqm
                nc.scalar.activation(
                    ssqm[:], ssqm[:], mybir.ActivationFunctionType.Sqrt, scale=0.1
                )
                # acc[:, i] = sum_k emd * wgt
                nc.vector.tensor_mul(tv[:], ssqm[:], tv[:])
                nc.vector.tensor_reduce(
                    acc[:, i : i + 1], tv[:], axis=mybir.AxisListType.X, op=Alu.add
                )

            nc.sync.dma_start(out_d[:], acc[:])

    nc.compile()
    return nc


def _consts(k=K):
    F = k * C
    m01 = np.tile(np.array([0.0] + [1.0] * (C - 1), np.float16), k)
    mask_full = np.ascontiguousarray(np.broadcast_to(m01, (P, F)))

    # block-diagonal stationary, fp16-exact dyadic weights: for slot s,
    # class c: wst[10s+c, 2s] = (c+1)/16 -> Z1/16;
    #          wst[10s+c, 2s+1] = (c+1)^2/256 -> Z2/256
    wst = np.zeros((SLOT * C, 2 * SLOT), np.float16)
    wv1 = (np.arange(1, C + 1, dtype=np.float64) / 16.0).astype(np.float16)
    wv2 = (np.arange(1, C + 1, dtype=np.float64) ** 2 / 256.0).astype(np.float16)
    for s in range(SLOT):
        wst[10 * s : 10 * s + 10, 2 * s] = wv1
        wst[10 * s : 10 * s + 10, 2 * s + 1] = wv2

    ident = np.eye(P, dtype=np.float16)
    return mask_full, wst, ident


def _shards(x, per, shard_rows):
    out = []
    for i in range(NCORES):
        s = x[i * per : (i + 1) * per]
        pad = shard_rows - s.shape[0]
        if pad:
            s = np.concatenate([s, np.full((pad, C), PAD_VAL, x.dtype)], axis=0)
        out.append(np.ascontiguousarray(s.reshape(P, shard_rows // P, C)))
    return out


def kernel(p_target: np.ndarray, p_estimate: np.ndarray) -> np.ndarray:
    from concourse.bass_utils import run_bass_kernel_spmd

    if "nc" not in _CACHE:
        _CACHE["nc"] = _build_nc()
    nc = _CACHE["nc"]

    B = p_target.shape[0]
    per = B // NCORES
    mask_full, wst, ident = _consts()
    pt_sh = _shards(np.asarray(p_target).astype(np.float16), per, SHARD)
    pe_sh = _shards(np.asarray(p_estimate).astype(np.float16), per, SHARD)

    in_maps = [
        {
            "pt": pt_sh[i],
            "pe": pe_sh[i],
            "mask01": mask_full,
            "wst": wst,
            "ident": ident,
        }
        for i in range(NCORES)
    ]
    res = run_bass_kernel_spmd(nc, in_maps, core_ids=list(range(NCORES)))
    total = sum(
        res.results[i]["out"].astype(np.float64).sum() for i in range(NCORES)
    )
    return np.float32(total / B)


# revision 16
# speedup vs baseline: 1.7586x; 1.0136x over previous
"""Balanced EMD loss kernel for Trainium2 (8 NeuronCores, data parallel).

Math (per sample, classes w = 1..10):
    score = sum(pt * w);  var = sum(pt * (w - score)^2) = Z2 - Z1^2  (S0 ~= 1)
    cdf_diff = cumsum(pe) - cumsum(pt) = cumsum(pe - pt)
    emd = sqrt(mean(cdf_diff^2));  loss = sum(emd / var) / B

Layout: samples distributed over 128 partitions; each partition holds a
contiguous run of samples, 10 classes contiguous in the free dim.

Engine split per tile:
  VectorE: q = pe - pt; masked scan (per-sample cumsum via a periodic 0/1
           multiplicative reset pattern); per-sample reduce of cdf^2;
           small finishing ops (var, 1/var, loss accumulate).
  ScalarE: square of the cdf (in place) and PSUM->SBUF moves for the
           TensorE moment pipeline; final sqrt.
  TensorE: weighted moments Z1 = sum(pt*w), Z2 = sum(pt*w^2): transpose
           [128,120] chunks to class-on-partition, block-diagonal [120,24]
           matmul, transpose the [24,128] results back to a dense
           [128, samples*2] layout.
"""

import numpy as np

P = 128          # SBUF partitions
C = 10           # classes
K = 396          # samples per partition per tile (multiple of 12)
NT = 10          # tiles
KT = K * NT      # samples per partition per core
SHARD = P * KT   # padded rows per core
NCORES = 8
PAD_VAL = 0.1    # pt == pe == 0.1 -> emd == 0 -> zero loss contribution

SLOT = 12        # samples per transpose chunk (120 = SLOT*C free positions)
GCH = 3          # chunks per matmul group (PSUM bank holds 3*128 = 384 cols)

_CACHE = {}


def _build_nc(k=K, nt=NT):
    import concourse.bass as bass
    import concourse.tile as tile
    from concourse import bacc, mybir

    dt = mybir.dt.float32
    dth = mybir.dt.float16
    Alu = mybir.AluOpType
    F = k * C
    n_chunk = k // SLOT              # transpose chunks per tile
    n_group = n_chunk // GCH         # matmul groups per tile
    CW = SLOT * C                    # 120 free positions per chunk
    GW = GCH * P                     # matmul group column count (384)
    MW = GCH * 2 * SLOT              # momd free elems per group (72)

    nc = bacc.Bacc("TRN2")
    pt_d = nc.dram_tensor("pt", [P, k * nt, C], dth, kind="ExternalInput").ap()
    pe_d = nc.dram_tensor("pe", [P, k * nt, C], dth, kind="ExternalInput").ap()
    msk_d = nc.dram_tensor("mask01", [P, F], dth, kind="ExternalInput").ap()
    w_d = nc.dram_tensor("wst", [CW, 2 * SLOT], dth, kind="ExternalInput").ap()
    id_d = nc.dram_tensor("ident", [P, P], dth, kind="ExternalInput").ap()
    out_d = nc.dram_tensor("out", [P, nt + 1], dt, kind="ExternalOutput").ap()

    with tile.TileContext(nc) as tc:
        with (
            tc.tile_pool(name="consts", bufs=1) as cpool,
            tc.tile_pool(name="ins", bufs=3) as ipool,
            tc.tile_pool(name="mm", bufs=3) as mpool,
            tc.tile_pool(name="small", bufs=2) as spool,
            tc.tile_pool(name="ps1", bufs=2, space="PSUM") as ppool1,
            tc.tile_pool(name="ps2", bufs=2, space="PSUM") as ppool2,
            tc.tile_pool(name="ps3", bufs=2, space="PSUM") as ppool3,
            tc.tile_pool(name="outp", bufs=1) as opool,
        ):
            # tile schedule: two warmup half-tiles shorten the initial DVE
            # stall; their input DMAs are issued before the const DMAs
            k1 = (k // 2 // SLOT) * SLOT
            if k1 >= SLOT and k - k1 >= SLOT:
                tiles = [(0, k1), (k1, k - k1)]
            else:
                tiles = [(0, k)]
            off0 = tiles[-1][0] + tiles[-1][1]
            tiles += [(o, k) for o in range(off0, k * nt, k)]

            def load(off, ki):
                ptt = ipool.tile([P, F], dth, tag="ptt")
                nc.sync.dma_start(
                    ptt[:, : ki * C].rearrange("p (k c) -> p k c", c=C),
                    pt_d[:, off : off + ki, :],
                )
                pet = ipool.tile([P, F], dth, tag="pet")
                nc.sync.dma_start(
                    pet[:, : ki * C].rearrange("p (k c) -> p k c", c=C),
                    pe_d[:, off : off + ki, :],
                )
                return ptt, pet

            preload = load(*tiles[0])

            cmask = cpool.tile([P, F], dth, tag="cmask")
            nc.sync.dma_start(cmask[:], msk_d[:])
            wst = cpool.tile([CW, 2 * SLOT], dth, tag="wst")
            nc.sync.dma_start(wst[:], w_d[:])
            ident = cpool.tile([P, P], dth, tag="ident")
            nc.sync.dma_start(ident[:], id_d[:])

            acc = opool.tile([P, len(tiles)], dt, tag="acc")

            for i, (off, ki) in enumerate(tiles):
                fi = ki * C
                ptt, pet = preload if i == 0 else load(off, ki)

                # ---- VectorE cdf pipeline ----
                # q = pe - pt  (in place on the pe tile)
                nc.vector.tensor_sub(pet[:, :fi], pet[:, :fi], ptt[:, :fi])
                # per-sample cumsum: state = mask*state + q, in place
                nc.vector.tensor_tensor_scan(
                    pet[:, :fi], cmask[:, :fi], pet[:, :fi], 0.0,
                    op0=Alu.mult, op1=Alu.add,
                )
                # square on the scalar engine, in place
                nc.scalar.square(pet[:, :fi], pet[:, :fi])
                # ssq = sum over classes of cdf_diff^2
                ssqm = spool.tile([P, k], dt, tag="ssqm")
                nc.vector.tensor_reduce(
                    ssqm[:, :ki],
                    pet[:, :fi].rearrange("p (k c) -> p k c", c=C),
                    axis=mybir.AxisListType.X,
                    op=Alu.add,
                )

                # ---- TensorE moment pipeline over pt ----
                # transpose [128,120] chunks to class-on-partition, then
                # matmul with the chunk as STATIONARY and the block-diag
                # weight matrix as moving: out = sb_chunk^T @ wst =
                # [128 samples, 24] -- moments, already dense.
                nchk = ki // SLOT
                ngrp = (nchk + GCH - 1) // GCH
                n_half = (nchk + 1) // 2  # chunks in first PSUM bank
                nha = (n_chunk + 1) // 2  # max bank-a chunk capacity
                momd = mpool.tile([P, 2 * k], dt, tag="momd")
                mdp_a = ppool2.tile([P, nha * 2 * SLOT], dt, tag="mdp_a")
                mdp_b = ppool3.tile(
                    [P, (n_chunk - nha) * 2 * SLOT], dt, tag="mdp_b"
                )
                for g in range(ngrp):
                    gch = min(GCH, nchk - g * GCH)
                    pst = ppool1.tile([CW, GW], dth, tag="pst")
                    for j in range(gch):
                        ch = g * GCH + j
                        nc.tensor.transpose(
                            pst[:, bass.ts(j, P)],
                            ptt[:, bass.ts(ch, CW)],
                            ident[:],
                        )
                    sb = mpool.tile([CW, GW], dth, tag="sb")
                    nc.scalar.copy(sb[:, : gch * P], pst[:, : gch * P])
                    for j in range(gch):
                        ch = g * GCH + j
                        dst = (
                            mdp_a[:, bass.ts(ch, 2 * SLOT)]
                            if ch < n_half
                            else mdp_b[:, bass.ts(ch - n_half, 2 * SLOT)]
                        )
                        nc.tensor.matmul(
                            dst, sb[:, bass.ts(j, P)], wst[:],
                            start=True, stop=True,
                        )
                nc.scalar.copy(
                    momd[:, : n_half * 2 * SLOT], mdp_a[:, : n_half * 2 * SLOT]
                )
                if nchk > n_half:
                    nc.scalar.copy(
                        momd[:, n_half * 2 * SLOT : nchk * 2 * SLOT],
                        mdp_b[:, : (nchk - n_half) * 2 * SLOT],
                    )

                # ---- finishing ----
                # momd free layout: (chunk, slot, mtype) -> sample index
                # 12*chunk + slot; mtype 0 -> Z1/16, 1 -> Z2/256
                z1 = momd[:, : 2 * ki].rearrange("p (k m) -> p k m", m=2)[:, :, 0]
                z2 = momd[:, : 2 * ki].rearrange("p (k m) -> p k m", m=2)[:, :, 1]
                tv = spool.tile([P, k], dt, tag="tv")
                # var = 256*z2 - 256*z1^2   (z1 = Z1/16, z2 = Z2/256)
                nc.vector.scalar_tensor_tensor(
                    tv[:, :ki], z1, -256.0, z1, op0=Alu.mult, op1=Alu.mult
                )
                nc.vector.scalar_tensor_tensor(
                    tv[:, :ki], z2, 256.0, tv[:, :ki], op0=Alu.mult, op1=Alu.add
                )
                nc.vector.reciprocal_approx_fast(tv[:, :ki], tv[:, :ki])
                # emd = sqrt(ssq/10), in place on ssqm
                nc.scalar.activation(
                    ssqm[:, :ki], ssqm[:, :ki],
                    mybir.ActivationFunctionType.Sqrt, scale=0.1,
                )
                # acc[:, i] = sum_k emd * wgt
                nc.vector.tensor_mul(tv[:, :ki], ssqm[:, :ki], tv[:, :ki])
                nc.vector.tensor_reduce(
                    acc[:, i : i + 1], tv[:, :ki],
                    axis=mybir.AxisListType.X, op=Alu.add,
                )

            nc.sync.dma_start(out_d[:, : len(tiles)], acc[:])

    nc.compile()
    return nc


def _consts(k=K):
    F = k * C
    m01 = np.tile(np.array([0.0] + [1.0] * (C - 1), np.float16), k)
    mask_full = np.ascontiguousarray(np.broadcast_to(m01, (P, F)))

    # block-diagonal stationary, fp16-exact dyadic weights: for slot s,
    # class c: wst[10s+c, 2s] = (c+1)/16 -> Z1/16;
    #          wst[10s+c, 2s+1] = (c+1)^2/256 -> Z2/256
    wst = np.zeros((SLOT * C, 2 * SLOT), np.float16)
    wv1 = (np.arange(1, C + 1, dtype=np.float64) / 16.0).astype(np.float16)
    wv2 = (np.arange(1, C + 1, dtype=np.float64) ** 2 / 256.0).astype(np.float16)
    for s in range(SLOT):
        wst[10 * s : 10 * s + 10, 2 * s] = wv1
        wst[10 * s : 10 * s + 10, 2 * s + 1] = wv2

    ident = np.eye(P, dtype=np.float16)
    return mask_full, wst, ident


def _shards(x, per, shard_rows):
    out = []
    for i in range(NCORES):
        s = x[i * per : (i + 1) * per]
        pad = shard_rows - s.shape[0]
        if pad:
            s = np.concatenate([s, np.full((pad, C), PAD_VAL, x.dtype)], axis=0)
        out.append(np.ascontiguousarray(s.reshape(P, shard_rows // P, C)))
    return out


def kernel(p_target: np.ndarray, p_estimate: np.ndarray) -> np.ndarray:
    from concourse.bass_utils import run_bass_kernel_spmd

    if "nc" not in _CACHE:
        _CACHE["nc"] = _build_nc()
    nc = _CACHE["nc"]

    B = p_target.shape[0]
    per = B // NCORES
    mask_full, wst, ident = _consts()
    pt_sh = _shards(np.asarray(p_target).astype(np.float16), per, SHARD)
    pe_sh = _shards(np.asarray(p_estimate).astype(np.float16), per, SHARD)

    in_maps = [
        {
            "pt": pt_sh[i],
            "pe": pe_sh[i],
            "mask01": mask_full,
            "wst": wst,
            "ident": ident,
        }
        for i in range(NCORES)
    ]
    res = run_bass_kernel_spmd(nc, in_maps, core_ids=list(range(NCORES)))
    total = sum(
        res.results[i]["out"].astype(np.float64).sum() for i in range(NCORES)
    )
    return np.float32(total / B)


# revision 17
# speedup vs baseline: 1.8332x; 1.0424x over previous
"""Balanced EMD loss kernel for Trainium2 (8 NeuronCores, data parallel).

Math (per sample, classes w = 1..10):
    score = sum(pt * w);  var = sum(pt * (w - score)^2) = Z2 - Z1^2  (S0 ~= 1)
    cdf_diff = cumsum(pe) - cumsum(pt) = cumsum(pe - pt)
    emd = sqrt(mean(cdf_diff^2));  loss = sum(emd / var) / B

Layout: samples distributed over 128 partitions; each partition holds a
contiguous run of samples, 10 classes contiguous in the free dim.

Engine split per tile:
  VectorE: q = pe - pt; masked scan (per-sample cumsum via a periodic 0/1
           multiplicative reset pattern); per-sample reduce of cdf^2;
           small finishing ops (var, 1/var, loss accumulate).
  ScalarE: square of the cdf (in place) and PSUM->SBUF moves for the
           TensorE moment pipeline; final sqrt.
  TensorE: weighted moments Z1 = sum(pt*w), Z2 = sum(pt*w^2): transpose
           [128,120] chunks to class-on-partition, block-diagonal [120,24]
           matmul, transpose the [24,128] results back to a dense
           [128, samples*2] layout.
"""

import numpy as np

P = 128          # SBUF partitions
C = 10           # classes
K = 396          # samples per partition per tile (multiple of 12)
NT = 10          # tiles
KT = K * NT      # samples per partition per core
SHARD = P * KT   # padded rows per core
NCORES = 8
PAD_VAL = 0.1    # pt == pe == 0.1 -> emd == 0 -> zero loss contribution

SLOT = 12        # samples per transpose chunk (120 = SLOT*C free positions)
GCH = 3          # chunks per matmul group (PSUM bank holds 3*128 = 384 cols)

_CACHE = {}


def _build_nc(k=K, nt=NT):
    import concourse.bass as bass
    import concourse.tile as tile
    from concourse import bacc, mybir

    dt = mybir.dt.float32
    dth = mybir.dt.float16
    Alu = mybir.AluOpType
    F = k * C
    n_chunk = k // SLOT              # transpose chunks per tile
    n_group = n_chunk // GCH         # matmul groups per tile
    CW = SLOT * C                    # 120 free positions per chunk
    GW = GCH * P                     # matmul group column count (384)
    MW = GCH * 2 * SLOT              # momd free elems per group (72)

    nc = bacc.Bacc("TRN2")
    pt_d = nc.dram_tensor("pt", [P, k * nt, C], dth, kind="ExternalInput").ap()
    pe_d = nc.dram_tensor("pe", [P, k * nt, C], dth, kind="ExternalInput").ap()
    msk_d = nc.dram_tensor("mask01", [P, F], dth, kind="ExternalInput").ap()
    w_d = nc.dram_tensor("wst", [CW, 2 * SLOT], dth, kind="ExternalInput").ap()
    id_d = nc.dram_tensor("ident", [P, P], dth, kind="ExternalInput").ap()
    out_d = nc.dram_tensor("out", [P, nt + 1], dt, kind="ExternalOutput").ap()

    with tile.TileContext(nc) as tc:
        with (
            tc.tile_pool(name="consts", bufs=1) as cpool,
            tc.tile_pool(name="ins", bufs=4) as ipool,
            tc.tile_pool(name="mm", bufs=4) as mpool,
            tc.tile_pool(name="small", bufs=3) as spool,
            tc.tile_pool(name="ps1", bufs=4, space="PSUM") as ppool1,
            tc.tile_pool(name="ps2", bufs=2, space="PSUM") as ppool2,
            tc.tile_pool(name="ps3", bufs=2, space="PSUM") as ppool3,
            tc.tile_pool(name="outp", bufs=1) as opool,
        ):
            # tile schedule: two warmup half-tiles shorten the initial DVE
            # stall; their input DMAs are issued before the const DMAs
            k1 = (k // 2 // SLOT) * SLOT
            if k1 >= SLOT and k - k1 >= SLOT:
                tiles = [(0, k1), (k1, k - k1)]
            else:
                tiles = [(0, k)]
            off0 = tiles[-1][0] + tiles[-1][1]
            tiles += [(o, k) for o in range(off0, k * nt, k)]

            def load(off, ki):
                ptt = ipool.tile([P, F], dth, tag="ptt")
                nc.sync.dma_start(
                    ptt[:, : ki * C].rearrange("p (k c) -> p k c", c=C),
                    pt_d[:, off : off + ki, :],
                )
                pet = ipool.tile([P, F], dth, tag="pet")
                nc.sync.dma_start(
                    pet[:, : ki * C].rearrange("p (k c) -> p k c", c=C),
                    pe_d[:, off : off + ki, :],
                )
                return ptt, pet

            preload = load(*tiles[0])

            cmask = cpool.tile([P, F], dth, tag="cmask")
            nc.sync.dma_start(cmask[:], msk_d[:])
            wst = cpool.tile([CW, 2 * SLOT], dth, tag="wst")
            nc.sync.dma_start(wst[:], w_d[:])
            ident = cpool.tile([P, P], dth, tag="ident")
            nc.sync.dma_start(ident[:], id_d[:])

            acc = opool.tile([P, len(tiles)], dt, tag="acc")

            for i, (off, ki) in enumerate(tiles):
                fi = ki * C
                ptt, pet = preload if i == 0 else load(off, ki)

                # ---- VectorE cdf pipeline ----
                # q = pe - pt  (in place on the pe tile)
                nc.vector.tensor_sub(pet[:, :fi], pet[:, :fi], ptt[:, :fi])
                # per-sample cumsum: state = mask*state + q, in place
                nc.vector.tensor_tensor_scan(
                    pet[:, :fi], cmask[:, :fi], pet[:, :fi], 0.0,
                    op0=Alu.mult, op1=Alu.add,
                )
                # square on the scalar engine, in place
                nc.scalar.square(pet[:, :fi], pet[:, :fi])
                # ssq = sum over classes of cdf_diff^2
                ssqm = spool.tile([P, k], dt, tag="ssqm")
                nc.vector.tensor_reduce(
                    ssqm[:, :ki],
                    pet[:, :fi].rearrange("p (k c) -> p k c", c=C),
                    axis=mybir.AxisListType.X,
                    op=Alu.add,
                )

                # ---- TensorE moment pipeline over pt ----
                # transpose [128,120] chunks to class-on-partition, then
                # matmul with the chunk as STATIONARY and the block-diag
                # weight matrix as moving: out = sb_chunk^T @ wst =
                # [128 samples, 24] -- moments, already dense.
                nchk = ki // SLOT
                ngrp = (nchk + GCH - 1) // GCH
                n_half = (nchk + 1) // 2  # chunks in first PSUM bank
                nha = (n_chunk + 1) // 2  # max bank-a chunk capacity
                momd = mpool.tile([P, 2 * k], dt, tag="momd")
                mdp_a = ppool2.tile([P, nha * 2 * SLOT], dt, tag="mdp_a")
                mdp_b = ppool3.tile(
                    [P, (n_chunk - nha) * 2 * SLOT], dt, tag="mdp_b"
                )
                for g in range(ngrp):
                    gch = min(GCH, nchk - g * GCH)
                    pst = ppool1.tile([CW, GW], dth, tag="pst")
                    for j in range(gch):
                        ch = g * GCH + j
                        nc.tensor.transpose(
                            pst[:, bass.ts(j, P)],
                            ptt[:, bass.ts(ch, CW)],
                            ident[:],
                        )
                    sb = mpool.tile([CW, GW], dth, tag="sb")
                    nc.scalar.copy(sb[:, : gch * P], pst[:, : gch * P])
                    for j in range(gch):
                        ch = g * GCH + j
                        dst = (
                            mdp_a[:, bass.ts(ch, 2 * SLOT)]
                            if ch < n_half
                            else mdp_b[:, bass.ts(ch - n_half, 2 * SLOT)]
                        )
                        nc.tensor.matmul(
                            dst, sb[:, bass.ts(j, P)], wst[:],
                            start=True, stop=True,
                        )
                nc.scalar.copy(
                    momd[:, : n_half * 2 * SLOT], mdp_a[:, : n_half * 2 * SLOT]
                )
                if nchk > n_half:
                    nc.scalar.copy(
                        momd[:, n_half * 2 * SLOT : nchk * 2 * SLOT],
                        mdp_b[:, : (nchk - n_half) * 2 * SLOT],
                    )

                # ---- finishing ----
                # momd free layout: (chunk, slot, mtype) -> sample index
                # 12*chunk + slot; mtype 0 -> Z1/16, 1 -> Z2/256
                z1 = momd[:, : 2 * ki].rearrange("p (k m) -> p k m", m=2)[:, :, 0]
                z2 = momd[:, : 2 * ki].rearrange("p (k m) -> p k m", m=2)[:, :, 1]
                tv = spool.tile([P, k], dt, tag="tv")
                # var = 256*z2 - 256*z1^2   (z1 = Z1/16, z2 = Z2/256)
                nc.vector.scalar_tensor_tensor(
                    tv[:, :ki], z1, -256.0, z1, op0=Alu.mult, op1=Alu.mult
                )
                nc.vector.scalar_tensor_tensor(
                    tv[:, :ki], z2, 256.0, tv[:, :ki], op0=Alu.mult, op1=Alu.add
                )
                nc.vector.reciprocal_approx_fast(tv[:, :ki], tv[:, :ki])
                # emd = sqrt(ssq/10), in place on ssqm
                nc.scalar.activation(
                    ssqm[:, :ki], ssqm[:, :ki],
                    mybir.ActivationFunctionType.Sqrt, scale=0.1,
                )
                # acc[:, i] = sum_k emd * wgt
                nc.vector.tensor_mul(tv[:, :ki], ssqm[:, :ki], tv[:, :ki])
                nc.vector.tensor_reduce(
                    acc[:, i : i + 1], tv[:, :ki],
                    axis=mybir.AxisListType.X, op=Alu.add,
                )

            nc.sync.dma_start(out_d[:, : len(tiles)], acc[:])

    nc.compile()
    return nc


def _consts(k=K):
    F = k * C
    m01 = np.tile(np.array([0.0] + [1.0] * (C - 1), np.float16), k)
    mask_full = np.ascontiguousarray(np.broadcast_to(m01, (P, F)))

    # block-diagonal stationary, fp16-exact dyadic weights: for slot s,
    # class c: wst[10s+c, 2s] = (c+1)/16 -> Z1/16;
    #          wst[10s+c, 2s+1] = (c+1)^2/256 -> Z2/256
    wst = np.zeros((SLOT * C, 2 * SLOT), np.float16)
    wv1 = (np.arange(1, C + 1, dtype=np.float64) / 16.0).astype(np.float16)
    wv2 = (np.arange(1, C + 1, dtype=np.float64) ** 2 / 256.0).astype(np.float16)
    for s in range(SLOT):
        wst[10 * s : 10 * s + 10, 2 * s] = wv1
        wst[10 * s : 10 * s + 10, 2 * s + 1] = wv2

    ident = np.eye(P, dtype=np.float16)
    return mask_full, wst, ident


def _shards(x, per, shard_rows):
    out = []
    for i in range(NCORES):
        s = x[i * per : (i + 1) * per]
        pad = shard_rows - s.shape[0]
        if pad:
            s = np.concatenate([s, np.full((pad, C), PAD_VAL, x.dtype)], axis=0)
        out.append(np.ascontiguousarray(s.reshape(P, shard_rows // P, C)))
    return out


def kernel(p_target: np.ndarray, p_estimate: np.ndarray) -> np.ndarray:
    from concourse.bass_utils import run_bass_kernel_spmd

    if "nc" not in _CACHE:
        _CACHE["nc"] = _build_nc()
    nc = _CACHE["nc"]

    B = p_target.shape[0]
    per = B // NCORES
    mask_full, wst, ident = _consts()
    pt_sh = _shards(np.asarray(p_target).astype(np.float16), per, SHARD)
    pe_sh = _shards(np.asarray(p_estimate).astype(np.float16), per, SHARD)

    in_maps = [
        {
            "pt": pt_sh[i],
            "pe": pe_sh[i],
            "mask01": mask_full,
            "wst": wst,
            "ident": ident,
        }
        for i in range(NCORES)
    ]
    res = run_bass_kernel_spmd(nc, in_maps, core_ids=list(range(NCORES)))
    total = sum(
        res.results[i]["out"].astype(np.float64).sum() for i in range(NCORES)
    )
    return np.float32(total / B)
